# revision 49
# baseline (speedup 1.0000x reference)
"""Trainium2 Bass kernel for the Camera ISP pipeline (mosaic -> gaussian blur
-> per-channel piecewise-linear calibration -> noise -> Malvar demosaic -> clip).

Strategy (per core, pure data parallel over batch: 4 images/core):
- Quad (polyphase) layout: bayer lattice split by (row parity, col parity).
- Row-decimated contiguous DMA loads (no 4B-strided DMA anywhere).
- Vertical blur + mosaic fused into banded matmuls on PE (fp32r).
- Horizontal blur + phase split on DVE with strided SBUF reads.
- 17-knot np.interp evaluated exactly in 8 custom DVE instructions
  (2 piecewise segments per instruction, knots rescaled to integers).
- Malvar 5x5 demosaic as banded matmuls on PE accumulating in PSUM.
- Clip/assembly: ACT relu (PSUM->SBUF) + GPSIMD min, writing interleaved
  full-width rows; contiguous row-granular DMA out.
"""

import sys

sys.path.insert(0, "/opt/trn_rl_repo")

import numpy as np

import concourse.bacc as bacc
import concourse.bass as bass
import concourse.tile as tile
from concourse import mybir
from concourse.bass_utils import run_bass_kernel_spmd
from concourse import dve_ops as _dops
from concourse.dve_spec import (
    C0, C1, C2, C3, One, Spec, Src0, Src1,
    _has_src1, _spill_c3_to_src1, lower, relu,
)
from concourse.dve_uop import DveOpSpec

F32 = mybir.dt.float32
F32R = mybir.dt.float32r
AOT = mybir.AluOpType
ACT_F = mybir.ActivationFunctionType

B_TOT, H, W = 32, 512, 512
N_CORES = 8
B_LOC = B_TOT // N_CORES           # images per core
Q = H // 2                          # quad-plane dim (256)
NPAGE = Q // 128                    # pages per quad plane (2)
DELTA = 255.0 / 16.0                # knot spacing of the LUT


# ---------------------------------------------------------------------------
# custom DVE ops (2 LUT segments per instruction)
# ---------------------------------------------------------------------------

def _head_ref(in0, in1, s0, s1, imm2):
    p = in0.shape[0]
    x = np.asarray(in0, np.float32).reshape(p, -1)
    d1 = np.asarray(in1, np.float32).reshape(p, 1)
    return (s0 * x + s1) + d1 * np.maximum(x - 1.0, 0.0)


def _pair_ref(in0, in1, s0, s1, imm2):
    x = np.asarray(in0, np.float32)
    acc = np.asarray(in1, np.float32).reshape(x.shape)
    return (acc + s0 * np.maximum(x - imm2, 0.0)) + s1 * np.maximum(
        x - imm2 - 1.0, 0.0
    )


def _register_op(name, spec):
    for op in _dops.OPS:
        if op.name == name:
            return op
    row = _dops._CUSTOM_DVE_ROW_BASE + len(_dops.OPS)
    assert row < 0x20, "custom DVE opcode rows exhausted"
    _dops._SUB_OPCODE_FOR_NAME[name] = row
    shas = {}
    for ver in ("v3", "v4"):
        try:
            s = DveOpSpec(name=name, opcode=row, uops=lower(spec, ver=ver),
                          rd1_en=_has_src1(spec))
            shas[ver] = s.sha(ver)
        except Exception:
            pass
    op = _dops.DveOp(name, spec, subdim=False, uops_sha=shas)
    _dops.OPS.append(op)
    _dops.CUSTOM_DVE_SPECS[name] = spec
    return op


# out = (C0*x + C1) + d1*relu(x - 1)   [affine + knot-1 term; d1 via in1]
INTERP_HEAD = _register_op(
    "CAM_INTERP_HEAD",
    Spec(body=_spill_c3_to_src1((C0 * Src0 + C1) + C3 * relu(Src0 - One)),
         reference=_head_ref),
)
# out = (acc + C0*relu(x - C2)) + C1*relu(x - C2 - 1)
INTERP_PAIR = _register_op(
    "CAM_INTERP_PAIR",
    Spec(body=(Src1 + C0 * relu(Src0 - C2)) + C1 * relu(Src0 - (C2 + One)),
         reference=_pair_ref),
)


# ---------------------------------------------------------------------------
# host-side constant planning
# ---------------------------------------------------------------------------

def _gauss1d(sigma=0.4):
    x = np.array([-1.0, 0.0, 1.0], dtype=np.float64)
    g = np.exp(-(x ** 2) / (2.0 * sigma ** 2))
    g /= g.sum()
    return g.astype(np.float32)  # [g0, g1, g0]


_G_AT = np.array([[0, 0, -1, 0, 0], [0, 0, 2, 0, 0], [-1, 2, 4, 2, -1],
                  [0, 0, 2, 0, 0], [0, 0, -1, 0, 0]], np.float32) / 8.0
_K_H = np.array([[0, 0, 0.5, 0, 0], [0, -1, 0, -1, 0], [-1, 4, 5, 4, -1],
                 [0, -1, 0, -1, 0], [0, 0, 0.5, 0, 0]], np.float32) / 8.0
_K_V = _K_H.T.copy()
_K_D = np.array([[0, 0, -1.5, 0, 0], [0, 2, 0, 2, 0], [-1.5, 0, 6, 0, -1.5],
                 [0, 2, 0, 2, 0], [0, 0, -1.5, 0, 0]], np.float32) / 8.0
FILTS = {"G": _G_AT, "H": _K_H, "V": _K_V, "D": _K_D}

# (out-quad (r,c), filter) -> output channel; raw channel per quad.
CONV_OUT = [
    ((0, 0), "V", 0), ((0, 1), "D", 0),
    ((0, 0), "H", 2), ((0, 1), "G", 1),
    ((1, 0), "G", 1), ((1, 1), "H", 0),
    ((1, 0), "D", 2), ((1, 1), "V", 2),
]
RAW_OUT = {(0, 0): 1, (0, 1): 2, (1, 0): 0, (1, 1): 1}  # quad -> raw channel
# input quad plane index: (pr, pc) -> 2*pr + pc
# image channel per quad (bayer gbrg): (0,0)=G1,(0,1)=B,(1,0)=R,(1,1)=G


class _BandBuilder:
    def __init__(self):
        self.mats = []          # list of [128,128] float32
        self._idx = {}

    def add(self, m):
        key = m.tobytes()
        if key not in self._idx:
            self._idx[key] = len(self.mats)
            self.mats.append(m.copy())
        return self._idx[key]


def build_plan(yp):
    """All host-derived constants. yp: [3,17] float32 (255-domain)."""
    yp = np.asarray(yp, np.float32)
    g = _gauss1d()
    g0, g1 = float(g[0]), float(g[1])
    scale_v = 255.0 * g1 / DELTA          # folded into v-blur bands
    rho = g0 / g1                          # h-blur neighbor weight

    bb = _BandBuilder()

    def reflected_src(i_src, pr):
        """Quad-row index after reflect padding (parity preserved)."""
        if 0 <= i_src < Q:
            return i_src
        y_src = 2 * i_src + pr
        y_r = -y_src if y_src < 0 else 2 * (H - 1) - y_src
        assert y_r % 2 == pr
        return y_r // 2

    def emit_groups(groups, page):
        """groups: (plane, sj) -> [(si, w, pr)]. Returns matmul list
        [(band_idx, plane, src_page, sj)] for one 128-row out page, with
        cross-page and reflect terms folded into one-hot band entries."""
        mats = {}
        for (plane, sj), terms in sorted(groups.items()):
            for m in range(128):
                i_out = 128 * page + m
                for si, w, pr in terms:
                    i_src = reflected_src(i_out + si, pr)
                    sp, k = i_src // 128, i_src % 128
                    key = (plane, sp, sj)
                    if key not in mats:
                        mats[key] = np.zeros((128, 128), np.float32)
                    mats[key][k, m] += w
        return [(bb.add(mat), plane, sp, sj)
                for (plane, sp, sj), mat in sorted(mats.items(),
                                                   key=lambda x: x[0])]

    # vb fams: (name, center plane, neighbor plane, direction)
    # planes by id: 0=Gev 1=God 2=Rod 3=Bev
    # "up": out[i] = cd*cen[i] + co*(nei[i-1] + nei[i])   (even out rows)
    # "dn": out[i] = cd*cen[i] + co*(nei[i] + nei[i+1])   (odd out rows)
    cd, co = scale_v * g1, scale_v * g0
    vb_plan = [
        ("eA", 0, 2, "up", 0), ("eB", 3, 1, "up", 0),
        ("oA", 2, 0, "dn", 1), ("oB", 1, 3, "dn", 1),
    ]
    vb_mm = {}
    for name, cen, nei, d, r in vb_plan:
        nei_pr = 1 - r
        groups = {(cen, 0): [(0, cd, r)]}
        if d == "up":
            groups[(nei, 0)] = [(-1, co, nei_pr), (0, co, nei_pr)]
        else:
            groups[(nei, 0)] = [(0, co, nei_pr), (1, co, nei_pr)]
        vb_mm[name] = [emit_groups(groups, page) for page in range(NPAGE)]

    # ---- demosaic bands ----------------------------------------------
    dem = []
    for (r, c), fname, ch in CONV_OUT:
        K = FILTS[fname]
        groups = {}
        for dy in range(-2, 3):
            for dx in range(-2, 3):
                w = float(K[2 + dy, 2 + dx])
                if w == 0.0:
                    continue
                pr = (r + dy) % 2
                si = (r + dy - pr) // 2
                pc = (c + dx) % 2
                sj = (c + dx - pc) // 2
                plane = 2 * pr + pc
                groups.setdefault((plane, sj), []).append((si, w, pr))
        pages = [emit_groups(groups, page) for page in range(NPAGE)]
        dem.append(((r, c), fname, ch, pages))

    # ---- interp coefficients (output scale 1/255) ---------------------
    yps = yp / 255.0
    interp = []
    for ch in range(3):
        y0 = float(yps[ch, 0])
        s = np.diff(yps[ch]).astype(np.float64)   # 16 slopes per knot-unit
        d = np.diff(s)                            # 15 kink deltas (i=1..15)
        interp.append({
            "y0": y0, "s0": float(s[0]),
            "d": [float(v) for v in d],           # d[i-1] = kink at knot i
        })

    bands = np.stack(bb.mats).astype(np.float32)
    vb_idx = sorted({e[0] for mm in vb_mm.values() for page in mm
                     for e in [page] for e in page} | set())
    vb_used = sorted({e[0] for pages in vb_mm.values()
                      for page in pages for e in page})
    return {
        "bands": bands, "vb_used": vb_used,
        "vb_mm": vb_mm, "dem": dem, "interp": interp, "rho": rho,
    }


# ---------------------------------------------------------------------------
# kernel builder
# ---------------------------------------------------------------------------

def _row_decimated(ap2d, parity):
    """[512, 512] DRAM AP -> [128, NPAGE, 512] for rows parity::2."""
    rows = ap2d.rearrange("(r two) w -> two r w", two=2)[parity]
    return rows.rearrange("(q p) w -> p q w", p=128)


def build_kernel(plan):
    nc = bacc.Bacc(None, target_bir_lowering=False, debug=False)
    im = nc.dram_tensor("im", [B_LOC, 3, H, W], F32R,
                        kind="ExternalInput").ap()
    noise = nc.dram_tensor("noise", [B_LOC, 1, H, W], F32,
                           kind="ExternalInput").ap()
    nbands = plan["bands"].shape[0]
    bands_d = nc.dram_tensor("bands", [nbands, 128, 128], F32R,
                             kind="ExternalInput").ap()
    out = nc.dram_tensor("out", [B_LOC, 3, H, W], F32,
                         kind="ExternalOutput").ap()

    rho = plan["rho"]
    itp = plan["interp"]
    quad_ch = {0: 1, 1: 2, 2: 0, 3: 1}   # quad idx -> lut channel (G,B,R,G)

    from contextlib import ExitStack
    with tile.TileContext(nc) as tc, ExitStack() as ctx:
        consts = ctx.enter_context(tc.tile_pool(name="consts", bufs=1))
        imp = ctx.enter_context(tc.tile_pool(name="imp", bufs=2))
        nsp = ctx.enter_context(tc.tile_pool(name="nsp", bufs=2))
        vbp = ctx.enter_context(tc.tile_pool(name="vbp", bufs=3))
        sxp = ctx.enter_context(tc.tile_pool(name="sxp", bufs=2))
        xtp = ctx.enter_context(tc.tile_pool(name="xtp", bufs=2))
        accp = ctx.enter_context(tc.tile_pool(name="accp", bufs=2))
        nyr = ctx.enter_context(tc.tile_pool(name="nyr", bufs=2))
        cvp = ctx.enter_context(tc.tile_pool(name="cvp", bufs=4))
        outp = ctx.enter_context(tc.tile_pool(name="outp", bufs=1))
        psum_vb = ctx.enter_context(
            tc.tile_pool(name="psvb", bufs=2, space="PSUM"))
        psum_dm = ctx.enter_context(
            tc.tile_pool(name="psdm", bufs=6, space="PSUM"))

        if True:
            # --- constants (vblur bands first, then the rest) ---
            bands_all = consts.tile([128, nbands, 128], F32R, tag="bands")
            bsrc = bands_d.rearrange("n k m -> k n m")
            vb_used = plan["vb_used"]
            nvb = max(vb_used) + 1 if vb_used else 0
            nc.sync.dma_start(out=bands_all[:, 0:nvb, :],
                              in_=bsrc[:, 0:nvb, :])
            band_t = [bands_all[:, b, :] for b in range(nbands)]

            def load_rest_bands():
                if nvb < nbands:
                    nc.sync.dma_start(out=bands_all[:, nvb:nbands, :],
                                      in_=bsrc[:, nvb:nbands, :])
            d1_t = consts.tile([128, 3], F32, tag="d1")
            for ch in range(3):
                nc.vector.memset(d1_t[:, ch:ch + 1], itp[ch]["d"][0])

            def emit_front(b, mid_cb=None):
                # loads (row-decimated, contiguous rows) + v-blur (PE) + evac
                # planes: 0=Gev 1=God 2=Rod 3=Bev
                plane_srcs = [(1, 0), (1, 1), (0, 1), (2, 0)]
                ptiles = [None] * 4
                for pi in (3, 1, 0, 2):
                    ch, par = plane_srcs[pi]
                    t = imp.tile([128, NPAGE, W], F32R, tag=f"plane{pi}",
                                 name=f"pl{b}_{pi}")
                    nc.sync.dma_start(out=t,
                                      in_=_row_decimated(im[b, ch], par))
                    ptiles[pi] = t
                if mid_cb is not None:
                    mid_cb()
                ntiles = []
                for par in range(2):
                    t = nsp.tile([128, NPAGE, W], F32, tag=f"noise{par}",
                                 name=f"ns{b}_{par}")
                    nc.sync.dma_start(out=t,
                                      in_=_row_decimated(noise[b, 0], par))
                    ntiles.append(t)
                vbt = {}
                for name in ("eB", "oB", "eA", "oA"):
                    vt = vbp.tile([128, NPAGE, W], F32, tag=f"vb{name}",
                                  name=f"vb{b}{name}")
                    for page, mm in enumerate(plan["vb_mm"][name]):
                        ps = psum_vb.tile([128, W], F32, tag="vbps",
                                          name=f"vps{b}{name}{page}")
                        for i, (bidx, plane, spage, _sj) in enumerate(mm):
                            nc.tensor.matmul(
                                ps[:], band_t[bidx],
                                ptiles[plane][:, spage, :],
                                start=(i == 0), stop=(i == len(mm) - 1))
                        nc.scalar.copy(out=vt[:, page, :], in_=ps[:])
                    vbt[name] = vt
                return vbt, ntiles

            fronts = [emit_front(0, mid_cb=load_rest_bands)]
            fronts.append(emit_front(1))
            for b in range(B_LOC):
                vbt, ntiles = fronts[b]
                if b + 2 < B_LOC:
                    fronts.append(emit_front(b + 2))

                # --- h-blur + phase split (DVE) -> x_tilde per quad ---
                # quads: 0:(0,0) 1:(0,1) 2:(1,0) 3:(1,1)
                xts = []
                for qi, (r, c) in enumerate(((0, 0), (0, 1), (1, 0), (1, 1))):
                    va = vbt["eA" if r == 0 else "oA"]   # valid even cols
                    vb_ = vbt["eB" if r == 0 else "oB"]  # valid odd cols
                    s = sxp.tile([128, NPAGE, Q], F32, tag="s",
                                 name=f"s{b}_{qi}")
                    xt = xtp.tile([128, NPAGE, Q], F32, tag=f"xt{qi}",
                                  name=f"xt{b}_{qi}")
                    if c == 0:
                        # s[j] = vb_odd[j-1] + vb_odd[j], s[0] = 2*vb_odd[0]
                        nc.gpsimd.tensor_add(
                            out=s[:, :, 1:Q],
                            in0=vb_[:, :, 1:2 * Q - 2:2],
                            in1=vb_[:, :, 3:2 * Q:2])
                        nc.vector.tensor_scalar_mul(
                            out=s[:, :, 0:1], in0=vb_[:, :, 1:2], scalar1=2.0)
                        cen = va[:, :, 0:2 * Q:2]
                    else:
                        # s[j] = va_even[j] + va_even[j+1], s[Q-1] = 2*va[2Q-2]
                        nc.gpsimd.tensor_add(
                            out=s[:, :, 0:Q - 1],
                            in0=va[:, :, 0:2 * Q - 3:2],
                            in1=va[:, :, 2:2 * Q - 1:2])
                        nc.vector.tensor_scalar_mul(
                            out=s[:, :, Q - 1:Q],
                            in0=va[:, :, 2 * Q - 2:2 * Q - 1], scalar1=2.0)
                        cen = vb_[:, :, 1:2 * Q:2]
                    nc.vector.scalar_tensor_tensor(
                        out=xt, in0=s, scalar=rho, in1=cen,
                        op0=AOT.mult, op1=AOT.add)
                    xts.append(xt)

                # --- output row tiles (per page, for early DMA-out) ---
                ot = {}
                for ch in range(3):
                    for r in range(2):
                        for pg in range(NPAGE):
                            ot[(ch, r, pg)] = outp.tile(
                                [128, W], F32, name=f"ot{ch}{r}{pg}",
                                tag=f"o{ch}{r}{pg}")

                # --- interp (custom DVE chains; G quads merged) + noise ---
                def interp_chain(ch, xflat, nels, tagc):
                    co = itp[ch]
                    a0 = accp.tile([128, nels], F32, tag="accA" + ("g" if tagc == "g" else ""),
                                   name=f"a0{b}{tagc}")
                    a1 = accp.tile([128, nels], F32, tag="accB" + ("g" if tagc == "g" else ""),
                                   name=f"a1{b}{tagc}")
                    nc.vector._custom_dve(
                        INTERP_HEAD, out=a0[:], in0=xflat,
                        in1=d1_t[:, ch:ch + 1],
                        s0=co["s0"], s1=co["y0"])
                    src, dst = a0, a1
                    for j in range(1, 8):
                        nc.vector._custom_dve(
                            INTERP_PAIR, out=dst[:], in0=xflat, in1=src[:],
                            s0=co["d"][2 * j - 1], s1=co["d"][2 * j],
                            imm2=float(2 * j))
                        src, dst = dst, src
                    return src

                acc_of = {}
                for qi in range(4):
                    a = interp_chain(
                        quad_ch[qi],
                        xts[qi][:].rearrange("p a b -> p (a b)"),
                        NPAGE * Q, "g" if qi in (0, 3) else "br")
                    acc_of[qi] = a[:]
                nyrtiles = []
                for qi in range(4):
                    npr = nyr.tile([128, NPAGE, Q + 2], F32R, tag=f"nyr{qi}",
                                   name=f"npr{b}_{qi}")
                    r, c = qi // 2, qi % 2
                    nc.gpsimd.tensor_add(
                        out=npr[:, :, 1:Q + 1],
                        in0=acc_of[qi].rearrange("p (a b) -> p a b", a=NPAGE),
                        in1=ntiles[r][:, :, c:2 * Q:2])
                    # reflect pad columns
                    lsrc = 1 + (1 if c == 0 else 0)
                    rsrc = 1 + (Q - 1 if c == 0 else Q - 2)
                    nc.gpsimd.tensor_copy(out=npr[:, :, 0:1],
                                          in_=npr[:, :, lsrc:lsrc + 1])
                    nc.gpsimd.tensor_copy(out=npr[:, :, Q + 1:Q + 2],
                                          in_=npr[:, :, rsrc:rsrc + 1])
                    nyrtiles.append(npr)
                    rch = RAW_OUT[(r, c)]
                    for pg in range(NPAGE):
                        nc.gpsimd.tensor_scalar(
                            out=ot[(rch, r, pg)][:, c:2 * Q:2],
                            in0=npr[:, pg, 1:Q + 1],
                            scalar1=0.0, scalar2=1.0,
                            op0=AOT.max, op1=AOT.min)


                # --- demosaic (PE) + clip (ACT relu + GPSIMD min) ---
                for (r, c), fname, ch, pages in plan["dem"]:
                    tcl = cvp.tile([128, NPAGE, Q], F32, tag="conv")
                    for page, mains in enumerate(pages):
                        ps = psum_dm.tile([128, Q], F32, tag="dmps")
                        for i, (bidx, plane, spage, sj) in enumerate(mains):
                            nc.tensor.matmul(
                                ps[:], band_t[bidx],
                                nyrtiles[plane][:, spage, 1 + sj:1 + sj + Q],
                                start=(i == 0), stop=(i == len(mains) - 1))
                        if b == B_LOC - 1:
                            nc.scalar.activation(out=tcl[:, page, :],
                                                 in_=ps[:], func=ACT_F.Relu)
                            nc.gpsimd.tensor_scalar_min(
                                out=ot[(ch, r, page)][:, c:2 * Q:2],
                                in0=tcl[:, page, :], scalar1=1.0)
                        else:
                            nc.scalar.activation(out=tcl[:, page, :],
                                                 in_=ps[:], func=ACT_F.Relu,
                                                 scale=-1.0, bias=1.0)
                            nc.scalar.activation(
                                out=ot[(ch, r, page)][:, c:2 * Q:2],
                                in_=tcl[:, page, :],
                                func=ACT_F.Relu, scale=-1.0, bias=1.0)

                # --- stores (per page) ---
                for ch in range(3):
                    for r in range(2):
                        dst = _row_decimated(out[b, ch], r)
                        for pg in range(NPAGE):
                            nc.sync.dma_start(out=dst[:, pg, :],
                                              in_=ot[(ch, r, pg)][:])

    nc.compile()
    return nc


RAW_OUT_IDX = [(0, 1), (1, 2), (2, 0), (3, 1)]  # (quad idx, raw channel)


# ---------------------------------------------------------------------------
# public entry
# ---------------------------------------------------------------------------

_CACHE = {}


def _get_compiled(yp):
    key = np.asarray(yp, np.float32).tobytes()
    if key not in _CACHE:
        plan = build_plan(yp)
        _CACHE[key] = (build_kernel(plan), plan)
    return _CACHE[key]


def kernel(im, yp, noise):
    im = np.ascontiguousarray(np.asarray(im, np.float32))
    yp = np.asarray(yp, np.float32)
    noise = np.asarray(noise, np.float32)
    nc, plan = _get_compiled(yp)
    noise_s = np.ascontiguousarray(noise * np.float32(1.0 / 255.0))
    in_maps = []
    for k in range(N_CORES):
        sl = slice(k * B_LOC, (k + 1) * B_LOC)
        in_maps.append({
            "im": im[sl],
            "noise": noise_s[sl],
            "bands": plan["bands"],
        })
    res = run_bass_kernel_spmd(nc, in_maps, core_ids=list(range(N_CORES)))
    return np.concatenate([r["out"] for r in res.results], axis=0)
